# revision 30
# baseline (speedup 1.0000x reference)
"""Trainium2 Bass kernel for nn_D_GA_1812476199112 (maxpool -> 16-head
attention over 1024 tokens -> proj -> batchnorm -> maxunpool).

Sharding: data-parallel over batch B=8, one batch element per NeuronCore.
Everything is local per core; no collectives.

Per-core pipeline (channels-on-partitions layout [C=64, N=1024]):
  1. MaxPool2d(2,2) via strided DVE max ops (pipelined with the x DMA in
     four quarters, behind PE-warmup dummy matmuls that keep the HAM clock
     hot); argmax becomes first-match masks (is_equal + not-found chain,
     matching jnp.argmax tie semantics) computed during attention idle.
  2. Q^T/K^T are produced directly in a "strip-packed" layout (head h of
     supergroup sg at partitions 32c..32c+3) using host-permuted zero-padded
     weight matrices; prep matmuls run 2x concurrent via row strips {0,64}.
     Q/K packs are stored as fp32r (full-rate fp32, ~12-bit mantissa): the
     rounding cancels in the softmax ratio (verified ~1e-4 end-to-end).
  3. Score matmuls compute S^T [keys, queries] (K=4, 3x concurrent via PE
     row tiling tile_position=(32c,0)) into PSUM chunks [128, 3*512];
     one ACT Exp per chunk (scale=0.5 folds the softmax scale; no max
     subtraction needed, |score| <~ 15). ACT is the bottleneck engine.
  4. AV matmuls in bf16 (softmax-ratio error cancellation keeps end-to-end
     error ~2e-4) with a ones-augmented V (V~ [128, 5] per head) so softmax
     denominators accumulate in PSUM row 32c+4 for free; col tiling
     (tile_position=(0,32c)) packs 4 heads into one 2-bank PSUM accumulator
     (start=False onto DVE-memset PSUM). The chunk loop is software-
     pipelined one stage (scores+exp emitted before the previous chunk's
     AV) so the in-order PE queue never blocks the next exp. PSUM budget:
     2x3-bank score slots + 2-bank accumulator = all 8 banks.
  5. Tail (pipelined by query-half): one-hot const matmuls (fp32r) gather
     denominators (em) and reorder o rows (gm) into (h,d) order; DVE
     reciprocal + multiply normalizes; proj matmul; BN folded into one ACT
     Identity(scale,bias); unpool via masked multiplies (3 DVE + 1 GpSimd).
"""
import numpy as np

DIM = 64
HEAD_DIM = 4
NUM_HEADS = 16
B = 8
H = W = 64
HP = WP = 32
N = HP * WP          # 1024 tokens
NKT = 8              # key tiles of 128
BN_EPS = 1e-5

_CACHE = {}


def _build_program():
    import concourse.bass as bass
    import concourse.mybir as mybir
    import concourse.tile as tile
    from concourse import bacc

    f32 = mybir.dt.float32
    f32r = mybir.dt.float32r
    bf16 = mybir.dt.bfloat16
    AF = mybir.ActivationFunctionType
    OP = mybir.AluOpType

    nc = bacc.Bacc("TRN2", debug=False)

    x_d = nc.dram_tensor("x", [DIM, H * W], f32, kind="ExternalInput").ap()
    wa_d = nc.dram_tensor("wa", [128, 512], f32r, kind="ExternalInput").ap()
    wb_d = nc.dram_tensor("wb", [128, 1154], f32, kind="ExternalInput").ap()
    out_d = nc.dram_tensor("out", [DIM, H * W], f32, kind="ExternalOutput").ap()

    with tile.TileContext(nc) as tc:
        with (
            tc.tile_pool(name="singles", bufs=1) as sg1,
            tc.tile_pool(name="expp", bufs=4) as expp,
        ):
            # hoist the ACT exp-table load to t=0 via a dummy exp
            warm = sg1.tile([1, 1], f32)
            nc.vector.memset(warm, 0.0)
            nc.scalar.activation(warm, warm, AF.Exp)

            # ---------- loads (2 packed weight DMAs + x in 2 halves) ----------
            x_sb = sg1.tile([DIM, H * W], f32)
            xr = x_sb.rearrange("p (i ti j tj) -> p i ti j tj", ti=2, tj=2, j=WP)
            for qq in range(4):
                nc.sync.dma_start(out=x_sb[:, qq * 1024:(qq + 1) * 1024],
                                  in_=x_d[:, qq * 1024:(qq + 1) * 1024])
            wa_sb = sg1.tile([128, 512], f32r)
            nc.sync.dma_start(out=wa_sb, in_=wa_d)
            wb_sb = sg1.tile([128, 1154], f32)
            nc.sync.dma_start(out=wb_sb, in_=wb_d)
            em_sb = [wa_sb[:, 64 * sg:64 * sg + 64] for sg in range(4)]
            gm_sb = [wa_sb[:, 256 + 64 * sg:256 + 64 * sg + 64] for sg in range(4)]
            wqp_sb = [wb_sb[0:64, 128 * sg:128 * sg + 128] for sg in range(4)]
            wkp2_sb = [wb_sb[64:128, 512 + 128 * sg:512 + 128 * sg + 128]
                       for sg in range(4)]
            wv_sb = wb_sb[0:64, 1024:1088]
            wv2_sb = wb_sb[64:128, 1024:1088]
            wproj_sb = wb_sb[0:64, 1088:1152]
            bns_sb = wb_sb[0:64, 1152:1153]
            bnb_sb = wb_sb[0:64, 1153:1154]

            # ---------- maxpool (per x-half) ----------
            m01 = sg1.tile([DIM, N], f32)
            m23 = sg1.tile([DIM, N], f32)
            pooled = sg1.tile([DIM, N], f32)
            m01r = m01.rearrange("p (i j) -> p i j", j=WP)
            m23r = m23.rearrange("p (i j) -> p i j", j=WP)
            pooledr = pooled.rearrange("p (i j) -> p i j", j=WP)
            v = [xr[:, :, 0, :, 0], xr[:, :, 0, :, 1],
                 xr[:, :, 1, :, 0], xr[:, :, 1, :, 1]]
            for hh in range(4):
                sl = slice(hh * 8, (hh + 1) * 8)
                nc.vector.tensor_tensor(m01r[:, sl], v[0][:, sl], v[1][:, sl], op=OP.max)
                nc.vector.tensor_tensor(m23r[:, sl], v[2][:, sl], v[3][:, sl], op=OP.max)
                nc.vector.tensor_tensor(pooledr[:, sl], m01r[:, sl], m23r[:, sl], op=OP.max)

            # ---------- qkv packs + V~ (per pooled-half) ----------
            ones16 = sg1.tile([128, 16], f32)
            nc.vector.memset(ones16, 1.0)
            qtp = [sg1.tile([128, N], f32r, tag=f"qtp{sg}", name=f"qtp{sg}") for sg in range(4)]
            ktp = [sg1.tile([128, N], f32r, tag=f"ktp{sg}", name=f"ktp{sg}") for sg in range(4)]
            vt = [sg1.tile([128, 16, 5], bf16, tag=f"vt{kt}", name=f"vt{kt}") for kt in range(NKT)]
            pooled2 = sg1.tile([128, N], f32)
            for qh in range(2):
                qsl = slice(qh * 512, (qh + 1) * 512)
                nc.vector.tensor_copy(pooled2[64:128, qsl], pooled[:, qsl])
            dummy_bf = sg1.tile([64, 512], bf16)
            nc.vector.memset(dummy_bf, 1.0)
            with (
                tc.tile_pool(name="prepq", bufs=6, space="PSUM") as prepq,
                tc.tile_pool(name="prepv", bufs=2, space="PSUM") as prepv,
            ):
                # PE warmup during the x DMA: back-to-back discarded matmuls
                # keep the HAM busy-window hot so prep matmuls run at 2.4GHz
                for wi in range(12):
                    w_ps = prepq.tile([128, 512], f32, tag="qkps")
                    nc.tensor.matmul(w_ps, dummy_bf[:, 0:128], dummy_bf,
                                     start=True, stop=True)
                for qh in range(2):
                    qsl = slice(qh * 512, (qh + 1) * 512)
                    for sg in range(4):
                        # q on row-strip 0 and k on strip 64 run concurrent;
                        # copies alternate ACT (early half) / DVE
                        cpq = nc.scalar.copy if qh == 0 else nc.vector.tensor_copy
                        cpk = nc.vector.tensor_copy
                        qt_ps = prepq.tile([128, 512], f32, tag="qkps")
                        nc.tensor.matmul(qt_ps, wqp_sb[sg], pooled[:, qsl],
                                         start=True, stop=True,
                                         tile_position=(0, 0))
                        kt_ps = prepq.tile([128, 512], f32, tag="qkps")
                        nc.tensor.matmul(kt_ps, wkp2_sb[sg], pooled2[64:128, qsl],
                                         start=True, stop=True,
                                         tile_position=(64, 0))
                        cpq(qtp[sg][:, qsl], qt_ps)
                        cpk(ktp[sg][:, qsl], kt_ps)
                    for kt in range(qh * 4, qh * 4 + 4):
                        v_ps = prepv.tile([128, DIM], f32, tag="vps")
                        if kt % 2 == 0:
                            nc.tensor.matmul(
                                v_ps, pooled[:, kt * 128:(kt + 1) * 128], wv_sb,
                                start=True, stop=True, tile_position=(0, 0))
                        else:
                            nc.tensor.matmul(
                                v_ps, pooled2[64:128, kt * 128:(kt + 1) * 128],
                                wv2_sb,
                                start=True, stop=True, tile_position=(64, 0))
                        nc.vector.tensor_copy(
                            vt[kt][:, :, 0:4],
                            v_ps.rearrange("p (h e) -> p h e", e=4))
                        nc.vector.tensor_copy(
                            vt[kt].rearrange("p h e -> p (h e)")[:, 4::5], ones16)

            # ---------- argmax masks (DVE, fills attention idle) ----------
            masks = []
            nf = None
            for p in range(4):
                eq = sg1.tile([DIM, N], f32, tag=f"eq{p}")
                eqr = eq.rearrange("p (i j) -> p i j", j=WP)
                nc.vector.tensor_tensor(eqr, v[p], pooledr, op=OP.is_equal)
                if p == 0:
                    masks.append(eq)
                    nf = sg1.tile([DIM, N], f32, tag="nf0")
                    nc.vector.tensor_scalar(nf, eq, -1.0, 1.0, op0=OP.mult, op1=OP.add)
                else:
                    mk = sg1.tile([DIM, N], f32, tag=f"mk{p}")
                    nc.vector.tensor_tensor(mk, eq, nf, op=OP.mult)
                    masks.append(mk)
                    if p < 3:
                        nf2 = sg1.tile([DIM, N], f32, tag=f"nf{p}")
                        nc.vector.tensor_tensor(nf2, nf, mk, op=OP.subtract)
                        nf = nf2

            # ---------- attention ----------
            chunks = [
                [(0, 0), (1, 0), (2, 0)],
                [(3, 0), (0, 1), (1, 1)],
                [(2, 1), (3, 1)],
            ]
            o_sb = [sg1.tile([128, N], f32r, tag=f"osb{sg}", name=f"osb{sg}") for sg in range(4)]
            with (
                tc.tile_pool(name="spsum", bufs=2, space="PSUM") as spsum,
                tc.tile_pool(name="opsum", bufs=1, space="PSUM") as opsum,
            ):
                for sg in range(4):
                    o_ps = opsum.tile([128, N], f32, tag="ops")
                    nc.vector.memset(o_ps, 0.0)
                    # software-pipeline by one chunk: emit scores+exp, then
                    # the PREVIOUS chunk's AV matmuls, so PE's in-order queue
                    # never parks AV work in front of the next chunk's scores
                    pend = None

                    def flush_av(pend):
                        kt0, ch0, e0 = pend
                        for i, (c, qh) in enumerate(ch0):
                            nc.tensor.matmul(
                                o_ps[32 * c:32 * c + 5,
                                     qh * 512:(qh + 1) * 512],
                                vt[kt0][:, 4 * sg + c, :],
                                e0[:, i * 512:(i + 1) * 512],
                                start=False, stop=(kt0 == NKT - 1),
                                skip_group_check=True,
                                tile_position=(0, 32 * c))

                    for kt in range(NKT):
                        for ch in chunks:
                            ncb = len(ch)
                            s_ps = spsum.tile([128, 3 * 512], f32, tag="slot")
                            for i, (c, qh) in enumerate(ch):
                                nc.tensor.matmul(
                                    s_ps[:, i * 512:(i + 1) * 512],
                                    ktp[sg][32 * c:32 * c + 4,
                                            kt * 128:(kt + 1) * 128],
                                    qtp[sg][32 * c:32 * c + 4,
                                            qh * 512:(qh + 1) * 512],
                                    start=True, stop=True,
                                    tile_position=(32 * c, 0))
                            e_sb = expp.tile([128, 3 * 512], bf16, tag="exp")
                            nc.scalar.activation(
                                e_sb[:, :ncb * 512], s_ps[:, :ncb * 512],
                                AF.Exp, scale=0.5)
                            if pend is not None:
                                flush_av(pend)
                            pend = (kt, ch, e_sb)
                    flush_av(pend)
                    # evict accumulator (o_ps fully initialized via memset)
                    nc.vector.tensor_copy(o_sb[sg][:, 0:512], o_ps[:, 0:512])
                    nc.vector.tensor_copy(o_sb[sg][:, 512:1024], o_ps[:, 512:1024])

            # ---------- tail: per-half pipeline normalize+proj+bn+unpool ----------
            out_sb = sg1.tile([DIM, H * W], f32)
            outr = out_sb.rearrange("p (i ti j tj) -> p i ti j tj",
                                    ti=2, tj=2, j=WP)
            yr_all = []
            with tc.tile_pool(name="tailps", bufs=1, space="PSUM") as tailps:
                dr = sg1.tile([DIM, N], f32)
                onorm = sg1.tile([DIM, N], f32)
                y = sg1.tile([DIM, N], f32)
                for qh in range(2):
                    qsl = slice(qh * 512, (qh + 1) * 512)
                    d_ps = tailps.tile([DIM, 512], f32, tag=f"dps{qh}")
                    o2_ps = tailps.tile([DIM, 512], f32, tag=f"o2ps{qh}")
                    for sg in range(4):
                        nc.tensor.matmul(
                            d_ps, em_sb[sg], o_sb[sg][:, qsl],
                            start=(sg == 0), stop=(sg == 3))
                    for sg in range(4):
                        nc.tensor.matmul(
                            o2_ps, gm_sb[sg], o_sb[sg][:, qsl],
                            start=(sg == 0), stop=(sg == 3))
                    nc.vector.reciprocal(dr[:, qsl], d_ps)
                    nc.vector.tensor_tensor(onorm[:, qsl], o2_ps, dr[:, qsl],
                                            op=OP.mult)
                    pj_ps = tailps.tile([DIM, 512], f32, tag=f"pjps{qh}")
                    nc.tensor.matmul(
                        pj_ps, wproj_sb, onorm[:, qsl],
                        start=True, stop=True)
                    nc.scalar.activation(
                        y[:, qsl], pj_ps, AF.Identity, bias=bnb_sb, scale=bns_sb)
                    yr = y.rearrange("p (i j) -> p i j", j=WP)
                    sl = slice(qh * 16, (qh + 1) * 16)
                    for p in range(4):
                        mr = masks[p].rearrange("p (i j) -> p i j", j=WP)
                        eng = nc.vector if p < 3 else nc.gpsimd
                        eng.tensor_tensor(
                            outr[:, sl, p // 2, :, p % 2], yr[:, sl], mr[:, sl],
                            op=OP.mult)
                    nc.sync.dma_start(
                        out=out_d[:, qh * 2048:(qh + 1) * 2048],
                        in_=out_sb[:, qh * 2048:(qh + 1) * 2048])

    nc.compile()
    return nc


def _host_inputs(x, w_qkv, w_proj, gamma, beta, bn_mean, bn_var):
    """Build the per-core input maps (host-side packing)."""
    wq = w_qkv[:, 0:64]
    wk = w_qkv[:, 64:128]
    wv = np.ascontiguousarray(w_qkv[:, 128:192], dtype=np.float32)
    wqp = np.zeros((4, DIM, 128), np.float32)
    wkp = np.zeros((4, DIM, 128), np.float32)
    em = np.zeros((4, 128, DIM), np.float32)
    gm = np.zeros((4, 128, DIM), np.float32)
    for sg in range(4):
        for c in range(4):
            h = 4 * sg + c
            for d in range(HEAD_DIM):
                wqp[sg][:, 32 * c + d] = wq[:, 4 * h + d]
                wkp[sg][:, 32 * c + d] = wk[:, 4 * h + d]
                gm[sg][32 * c + d, 4 * h + d] = 1.0
                em[sg][32 * c + 4, 4 * h + d] = 1.0
    inv = gamma / np.sqrt(bn_var + BN_EPS)
    bns = inv.reshape(DIM, 1).astype(np.float32)
    bnb = (beta - bn_mean * inv).reshape(DIM, 1).astype(np.float32)
    wproj = np.ascontiguousarray(w_proj, dtype=np.float32)

    wa = np.zeros((128, 512), np.float32)
    for sg in range(4):
        wa[:, 64 * sg:64 * sg + 64] = em[sg]
        wa[:, 256 + 64 * sg:256 + 64 * sg + 64] = gm[sg]
    wb = np.zeros((128, 1154), np.float32)
    for sg in range(4):
        wb[0:64, 128 * sg:128 * sg + 128] = wqp[sg]
        wb[0:64, 512 + 128 * sg:512 + 128 * sg + 128] = wkp[sg]
    wb[0:64, 1024:1088] = wv
    wb[0:64, 1088:1152] = wproj
    wb[0:64, 1152:1153] = bns
    wb[0:64, 1153:1154] = bnb
    wb[64:128, :] = wb[0:64, :]
    shared = {"wa": wa, "wb": wb}
    in_maps = []
    for b in range(B):
        m = dict(shared)
        m["x"] = np.ascontiguousarray(
            np.asarray(x)[b].reshape(DIM, H * W), dtype=np.float32)
        in_maps.append(m)
    return in_maps


def kernel(x, w_qkv, w_proj, gamma, beta, bn_mean, bn_var):
    from concourse import bass_utils

    if "nc" not in _CACHE:
        _CACHE["nc"] = _build_program()
    nc = _CACHE["nc"]
    in_maps = _host_inputs(
        np.asarray(x), np.asarray(w_qkv), np.asarray(w_proj),
        np.asarray(gamma), np.asarray(beta),
        np.asarray(bn_mean), np.asarray(bn_var))
    res = bass_utils.run_bass_kernel_spmd(nc, in_maps, core_ids=list(range(B)))
    out = np.stack([res.results[b]["out"].reshape(DIM, H, W) for b in range(B)])
    return out.astype(np.float32)



# revision 31
# speedup vs baseline: 1.0206x; 1.0206x over previous
"""Trainium2 Bass kernel for nn_D_GA_1812476199112 (maxpool -> 16-head
attention over 1024 tokens -> proj -> batchnorm -> maxunpool).

Sharding: data-parallel over batch B=8, one batch element per NeuronCore.
Everything is local per core; no collectives.

Per-core pipeline (channels-on-partitions layout [C=64, N=1024]):
  1. MaxPool2d(2,2) via strided DVE max ops (pipelined with the x DMA in
     four quarters, behind PE-warmup dummy matmuls that keep the HAM clock
     hot); argmax becomes first-match masks (is_equal + not-found chain,
     matching jnp.argmax tie semantics) computed during attention idle.
  2. Q^T/K^T are produced directly in a "strip-packed" layout (head h of
     supergroup sg at partitions 32c..32c+3) using host-permuted zero-padded
     weight matrices; prep matmuls run 2x concurrent via row strips {0,64}.
     Q/K packs are stored as fp32r (full-rate fp32, ~12-bit mantissa): the
     rounding cancels in the softmax ratio (verified ~1e-4 end-to-end).
  3. Score matmuls compute S^T [keys, queries] (K=4, 3x concurrent via PE
     row tiling tile_position=(32c,0)) into PSUM chunks [128, 3*512];
     one ACT Exp per chunk (scale=0.5 folds the softmax scale; no max
     subtraction needed, |score| <~ 15). ACT is the bottleneck engine.
  4. AV matmuls in bf16 (softmax-ratio error cancellation keeps end-to-end
     error ~2e-4) with a ones-augmented V (V~ [128, 5] per head) so softmax
     denominators accumulate in PSUM row 32c+4 for free; col tiling
     (tile_position=(0,32c)) packs 4 heads into one 2-bank PSUM accumulator
     (start=False onto DVE-memset PSUM). The chunk loop is software-
     pipelined one stage (scores+exp emitted before the previous chunk's
     AV) so the in-order PE queue never blocks the next exp. PSUM budget:
     2x3-bank score slots + 2-bank accumulator = all 8 banks.
  5. Tail (pipelined by query-half): one-hot const matmuls (fp32r) gather
     denominators (em) and reorder o rows (gm) into (h,d) order; DVE
     reciprocal + multiply normalizes; proj matmul; BN folded into one ACT
     Identity(scale,bias); unpool via masked multiplies (3 DVE + 1 GpSimd).
"""
import numpy as np

DIM = 64
HEAD_DIM = 4
NUM_HEADS = 16
B = 8
H = W = 64
HP = WP = 32
N = HP * WP          # 1024 tokens
NKT = 8              # key tiles of 128
BN_EPS = 1e-5

# Schraudolph exp (folds softmax scale 0.5), bf16/int16 flavor:
#   e^(0.5*s) ~= bitcast_bf16(int16(A_EXP*s + B_EXP))
A_EXP = float(0.5 * np.log2(np.e) * (1 << 7))
B_EXP = float(127 * (1 << 7) - 486411.0 / (1 << 16))
DVE_FRAC = 0.30      # fraction of exp chunks on DVE

_CACHE = {}


def _build_program():
    import concourse.bass as bass
    import concourse.mybir as mybir
    import concourse.tile as tile
    from concourse import bacc

    f32 = mybir.dt.float32
    f32r = mybir.dt.float32r
    bf16 = mybir.dt.bfloat16
    i16 = mybir.dt.int16
    AF = mybir.ActivationFunctionType
    OP = mybir.AluOpType

    nc = bacc.Bacc("TRN2", debug=False)

    x_d = nc.dram_tensor("x", [DIM, H * W], f32, kind="ExternalInput").ap()
    wa_d = nc.dram_tensor("wa", [128, 512], f32r, kind="ExternalInput").ap()
    wb_d = nc.dram_tensor("wb", [128, 1154], f32, kind="ExternalInput").ap()
    out_d = nc.dram_tensor("out", [DIM, H * W], f32, kind="ExternalOutput").ap()

    with tile.TileContext(nc) as tc:
        with (
            tc.tile_pool(name="singles", bufs=1) as sg1,
            tc.tile_pool(name="expp", bufs=4) as expp,
        ):
            # hoist the ACT exp-table load to t=0 via a dummy exp
            warm = sg1.tile([1, 1], f32)
            nc.vector.memset(warm, 0.0)
            nc.scalar.activation(warm, warm, AF.Exp)

            # ---------- loads (2 packed weight DMAs + x in 2 halves) ----------
            x_sb = sg1.tile([DIM, H * W], f32)
            xr = x_sb.rearrange("p (i ti j tj) -> p i ti j tj", ti=2, tj=2, j=WP)
            for qq in range(4):
                nc.sync.dma_start(out=x_sb[:, qq * 1024:(qq + 1) * 1024],
                                  in_=x_d[:, qq * 1024:(qq + 1) * 1024])
            wa_sb = sg1.tile([128, 512], f32r)
            nc.sync.dma_start(out=wa_sb, in_=wa_d)
            wb_sb = sg1.tile([128, 1154], f32)
            nc.sync.dma_start(out=wb_sb, in_=wb_d)
            em_sb = [wa_sb[:, 64 * sg:64 * sg + 64] for sg in range(4)]
            gm_sb = [wa_sb[:, 256 + 64 * sg:256 + 64 * sg + 64] for sg in range(4)]
            wqp_sb = [wb_sb[0:64, 128 * sg:128 * sg + 128] for sg in range(4)]
            wkp2_sb = [wb_sb[64:128, 512 + 128 * sg:512 + 128 * sg + 128]
                       for sg in range(4)]
            wv_sb = wb_sb[0:64, 1024:1088]
            wv2_sb = wb_sb[64:128, 1024:1088]
            wproj_sb = wb_sb[0:64, 1088:1152]
            bns_sb = wb_sb[0:64, 1152:1153]
            bnb_sb = wb_sb[0:64, 1153:1154]

            # ---------- maxpool (per x-half) ----------
            m01 = sg1.tile([DIM, N], f32)
            m23 = sg1.tile([DIM, N], f32)
            pooled = sg1.tile([DIM, N], f32)
            m01r = m01.rearrange("p (i j) -> p i j", j=WP)
            m23r = m23.rearrange("p (i j) -> p i j", j=WP)
            pooledr = pooled.rearrange("p (i j) -> p i j", j=WP)
            v = [xr[:, :, 0, :, 0], xr[:, :, 0, :, 1],
                 xr[:, :, 1, :, 0], xr[:, :, 1, :, 1]]
            for hh in range(4):
                sl = slice(hh * 8, (hh + 1) * 8)
                nc.vector.tensor_tensor(m01r[:, sl], v[0][:, sl], v[1][:, sl], op=OP.max)
                nc.vector.tensor_tensor(m23r[:, sl], v[2][:, sl], v[3][:, sl], op=OP.max)
                nc.vector.tensor_tensor(pooledr[:, sl], m01r[:, sl], m23r[:, sl], op=OP.max)

            # ---------- qkv packs + V~ (per pooled-half) ----------
            ones16 = sg1.tile([128, 16], f32)
            nc.vector.memset(ones16, 1.0)
            qtp = [sg1.tile([128, N], f32r, tag=f"qtp{sg}", name=f"qtp{sg}") for sg in range(4)]
            ktp = [sg1.tile([128, N], f32r, tag=f"ktp{sg}", name=f"ktp{sg}") for sg in range(4)]
            vt = [sg1.tile([128, 16, 5], bf16, tag=f"vt{kt}", name=f"vt{kt}") for kt in range(NKT)]
            pooled2 = sg1.tile([128, N], f32)
            for qh in range(2):
                qsl = slice(qh * 512, (qh + 1) * 512)
                nc.vector.tensor_copy(pooled2[64:128, qsl], pooled[:, qsl])
            dummy_bf = sg1.tile([64, 512], bf16)
            nc.vector.memset(dummy_bf, 1.0)
            with (
                tc.tile_pool(name="prepq", bufs=6, space="PSUM") as prepq,
                tc.tile_pool(name="prepv", bufs=2, space="PSUM") as prepv,
            ):
                # PE warmup during the x DMA: back-to-back discarded matmuls
                # keep the HAM busy-window hot so prep matmuls run at 2.4GHz
                for wi in range(12):
                    w_ps = prepq.tile([128, 512], f32, tag="qkps")
                    nc.tensor.matmul(w_ps, dummy_bf[:, 0:128], dummy_bf,
                                     start=True, stop=True)
                for qh in range(2):
                    qsl = slice(qh * 512, (qh + 1) * 512)
                    for sg in range(4):
                        # q on row-strip 0 and k on strip 64 run concurrent;
                        # copies alternate ACT (early half) / DVE
                        cpq = nc.scalar.copy if qh == 0 else nc.vector.tensor_copy
                        cpk = nc.vector.tensor_copy
                        qt_ps = prepq.tile([128, 512], f32, tag="qkps")
                        nc.tensor.matmul(qt_ps, wqp_sb[sg], pooled[:, qsl],
                                         start=True, stop=True,
                                         tile_position=(0, 0))
                        kt_ps = prepq.tile([128, 512], f32, tag="qkps")
                        nc.tensor.matmul(kt_ps, wkp2_sb[sg], pooled2[64:128, qsl],
                                         start=True, stop=True,
                                         tile_position=(64, 0))
                        cpq(qtp[sg][:, qsl], qt_ps)
                        cpk(ktp[sg][:, qsl], kt_ps)
                    for kt in range(qh * 4, qh * 4 + 4):
                        v_ps = prepv.tile([128, DIM], f32, tag="vps")
                        if kt % 2 == 0:
                            nc.tensor.matmul(
                                v_ps, pooled[:, kt * 128:(kt + 1) * 128], wv_sb,
                                start=True, stop=True, tile_position=(0, 0))
                        else:
                            nc.tensor.matmul(
                                v_ps, pooled2[64:128, kt * 128:(kt + 1) * 128],
                                wv2_sb,
                                start=True, stop=True, tile_position=(64, 0))
                        nc.vector.tensor_copy(
                            vt[kt][:, :, 0:4],
                            v_ps.rearrange("p (h e) -> p h e", e=4))
                        nc.vector.tensor_copy(
                            vt[kt].rearrange("p h e -> p (h e)")[:, 4::5], ones16)

            # ---------- argmax masks (DVE, fills attention idle) ----------
            masks = []
            nf = None
            for p in range(4):
                eq = sg1.tile([DIM, N], f32, tag=f"eq{p}")
                eqr = eq.rearrange("p (i j) -> p i j", j=WP)
                nc.vector.tensor_tensor(eqr, v[p], pooledr, op=OP.is_equal)
                if p == 0:
                    masks.append(eq)
                    nf = sg1.tile([DIM, N], f32, tag="nf0")
                    nc.vector.tensor_scalar(nf, eq, -1.0, 1.0, op0=OP.mult, op1=OP.add)
                else:
                    mk = sg1.tile([DIM, N], f32, tag=f"mk{p}")
                    nc.vector.tensor_tensor(mk, eq, nf, op=OP.mult)
                    masks.append(mk)
                    if p < 3:
                        nf2 = sg1.tile([DIM, N], f32, tag=f"nf{p}")
                        nc.vector.tensor_tensor(nf2, nf, mk, op=OP.subtract)
                        nf = nf2

            # ---------- attention ----------
            chunks = [
                [(0, 0), (1, 0), (2, 0)],
                [(3, 0), (0, 1), (1, 1)],
                [(2, 1), (3, 1)],
            ]
            o_sb = [sg1.tile([128, N], f32r, tag=f"osb{sg}", name=f"osb{sg}") for sg in range(4)]
            dve_acc = [0.0]
            with (
                tc.tile_pool(name="spsum", bufs=2, space="PSUM") as spsum,
                tc.tile_pool(name="opsum", bufs=1, space="PSUM") as opsum,
            ):
                for sg in range(4):
                    o_ps = opsum.tile([128, N], f32, tag="ops")
                    nc.vector.memset(o_ps, 0.0)
                    # software-pipeline by one chunk: emit scores+exp, then
                    # the PREVIOUS chunk's AV matmuls, so PE's in-order queue
                    # never parks AV work in front of the next chunk's scores
                    pend = None

                    def flush_av(pend):
                        kt0, ch0, e0 = pend
                        for i, (c, qh) in enumerate(ch0):
                            nc.tensor.matmul(
                                o_ps[32 * c:32 * c + 5,
                                     qh * 512:(qh + 1) * 512],
                                vt[kt0][:, 4 * sg + c, :],
                                e0[:, i * 512:(i + 1) * 512],
                                start=False, stop=(kt0 == NKT - 1),
                                skip_group_check=True,
                                tile_position=(0, 32 * c))

                    for kt in range(NKT):
                        for ch in chunks:
                            ncb = len(ch)
                            s_ps = spsum.tile([128, 3 * 512], f32, tag="slot")
                            for i, (c, qh) in enumerate(ch):
                                nc.tensor.matmul(
                                    s_ps[:, i * 512:(i + 1) * 512],
                                    ktp[sg][32 * c:32 * c + 4,
                                            kt * 128:(kt + 1) * 128],
                                    qtp[sg][32 * c:32 * c + 4,
                                            qh * 512:(qh + 1) * 512],
                                    start=True, stop=True,
                                    tile_position=(32 * c, 0))
                            e_sb = expp.tile([128, 3 * 512], bf16, tag="exp")
                            dve_acc[0] += DVE_FRAC
                            if dve_acc[0] >= 1.0:
                                dve_acc[0] -= 1.0
                                nc.vector.tensor_scalar(
                                    e_sb.bitcast(i16)[:, :ncb * 512],
                                    s_ps[:, :ncb * 512],
                                    A_EXP, B_EXP, op0=OP.mult, op1=OP.add)
                            else:
                                nc.scalar.activation(
                                    e_sb[:, :ncb * 512], s_ps[:, :ncb * 512],
                                    AF.Exp, scale=0.5)
                            if pend is not None:
                                flush_av(pend)
                            pend = (kt, ch, e_sb)
                    flush_av(pend)
                    # evict accumulator (o_ps fully initialized via memset)
                    nc.vector.tensor_copy(o_sb[sg][:, 0:512], o_ps[:, 0:512])
                    nc.vector.tensor_copy(o_sb[sg][:, 512:1024], o_ps[:, 512:1024])

            # ---------- tail: per-half pipeline normalize+proj+bn+unpool ----------
            out_sb = sg1.tile([DIM, H * W], f32)
            outr = out_sb.rearrange("p (i ti j tj) -> p i ti j tj",
                                    ti=2, tj=2, j=WP)
            yr_all = []
            with tc.tile_pool(name="tailps", bufs=1, space="PSUM") as tailps:
                dr = sg1.tile([DIM, N], f32)
                onorm = sg1.tile([DIM, N], f32)
                y = sg1.tile([DIM, N], f32)
                for qh in range(2):
                    qsl = slice(qh * 512, (qh + 1) * 512)
                    d_ps = tailps.tile([DIM, 512], f32, tag=f"dps{qh}")
                    o2_ps = tailps.tile([DIM, 512], f32, tag=f"o2ps{qh}")
                    for sg in range(4):
                        nc.tensor.matmul(
                            d_ps, em_sb[sg], o_sb[sg][:, qsl],
                            start=(sg == 0), stop=(sg == 3))
                    for sg in range(4):
                        nc.tensor.matmul(
                            o2_ps, gm_sb[sg], o_sb[sg][:, qsl],
                            start=(sg == 0), stop=(sg == 3))
                    nc.vector.reciprocal(dr[:, qsl], d_ps)
                    nc.vector.tensor_tensor(onorm[:, qsl], o2_ps, dr[:, qsl],
                                            op=OP.mult)
                    pj_ps = tailps.tile([DIM, 512], f32, tag=f"pjps{qh}")
                    nc.tensor.matmul(
                        pj_ps, wproj_sb, onorm[:, qsl],
                        start=True, stop=True)
                    nc.scalar.activation(
                        y[:, qsl], pj_ps, AF.Identity, bias=bnb_sb, scale=bns_sb)
                    yr = y.rearrange("p (i j) -> p i j", j=WP)
                    sl = slice(qh * 16, (qh + 1) * 16)
                    for p in range(4):
                        mr = masks[p].rearrange("p (i j) -> p i j", j=WP)
                        eng = nc.vector if p < 3 else nc.gpsimd
                        eng.tensor_tensor(
                            outr[:, sl, p // 2, :, p % 2], yr[:, sl], mr[:, sl],
                            op=OP.mult)
                    nc.sync.dma_start(
                        out=out_d[:, qh * 2048:(qh + 1) * 2048],
                        in_=out_sb[:, qh * 2048:(qh + 1) * 2048])

    nc.compile()
    return nc


def _host_inputs(x, w_qkv, w_proj, gamma, beta, bn_mean, bn_var):
    """Build the per-core input maps (host-side packing)."""
    wq = w_qkv[:, 0:64]
    wk = w_qkv[:, 64:128]
    wv = np.ascontiguousarray(w_qkv[:, 128:192], dtype=np.float32)
    wqp = np.zeros((4, DIM, 128), np.float32)
    wkp = np.zeros((4, DIM, 128), np.float32)
    em = np.zeros((4, 128, DIM), np.float32)
    gm = np.zeros((4, 128, DIM), np.float32)
    for sg in range(4):
        for c in range(4):
            h = 4 * sg + c
            for d in range(HEAD_DIM):
                wqp[sg][:, 32 * c + d] = wq[:, 4 * h + d]
                wkp[sg][:, 32 * c + d] = wk[:, 4 * h + d]
                gm[sg][32 * c + d, 4 * h + d] = 1.0
                em[sg][32 * c + 4, 4 * h + d] = 1.0
    inv = gamma / np.sqrt(bn_var + BN_EPS)
    bns = inv.reshape(DIM, 1).astype(np.float32)
    bnb = (beta - bn_mean * inv).reshape(DIM, 1).astype(np.float32)
    wproj = np.ascontiguousarray(w_proj, dtype=np.float32)

    wa = np.zeros((128, 512), np.float32)
    for sg in range(4):
        wa[:, 64 * sg:64 * sg + 64] = em[sg]
        wa[:, 256 + 64 * sg:256 + 64 * sg + 64] = gm[sg]
    wb = np.zeros((128, 1154), np.float32)
    for sg in range(4):
        wb[0:64, 128 * sg:128 * sg + 128] = wqp[sg]
        wb[0:64, 512 + 128 * sg:512 + 128 * sg + 128] = wkp[sg]
    wb[0:64, 1024:1088] = wv
    wb[0:64, 1088:1152] = wproj
    wb[0:64, 1152:1153] = bns
    wb[0:64, 1153:1154] = bnb
    wb[64:128, :] = wb[0:64, :]
    shared = {"wa": wa, "wb": wb}
    in_maps = []
    for b in range(B):
        m = dict(shared)
        m["x"] = np.ascontiguousarray(
            np.asarray(x)[b].reshape(DIM, H * W), dtype=np.float32)
        in_maps.append(m)
    return in_maps


def kernel(x, w_qkv, w_proj, gamma, beta, bn_mean, bn_var):
    from concourse import bass_utils

    if "nc" not in _CACHE:
        _CACHE["nc"] = _build_program()
    nc = _CACHE["nc"]
    in_maps = _host_inputs(
        np.asarray(x), np.asarray(w_qkv), np.asarray(w_proj),
        np.asarray(gamma), np.asarray(beta),
        np.asarray(bn_mean), np.asarray(bn_var))
    res = bass_utils.run_bass_kernel_spmd(nc, in_maps, core_ids=list(range(B)))
    out = np.stack([res.results[b]["out"].reshape(DIM, H, W) for b in range(B)])
    return out.astype(np.float32)



# revision 32
# speedup vs baseline: 1.0405x; 1.0196x over previous
"""Trainium2 Bass kernel for nn_D_GA_1812476199112 (maxpool -> 16-head
attention over 1024 tokens -> proj -> batchnorm -> maxunpool).

Sharding: data-parallel over batch B=8, one batch element per NeuronCore.
Everything is local per core; no collectives.

Per-core pipeline (channels-on-partitions layout [C=64, N=1024]):
  1. MaxPool2d(2,2) via strided DVE max ops (pipelined with the x DMA in
     four quarters, behind PE-warmup dummy matmuls that keep the HAM clock
     hot); argmax becomes first-match masks (is_equal + not-found chain,
     matching jnp.argmax tie semantics) computed during attention idle.
  2. Q^T/K^T are produced directly in a "strip-packed" layout (head h of
     supergroup sg at partitions 32c..32c+3) using host-permuted zero-padded
     weight matrices; prep matmuls run 2x concurrent via row strips {0,64}.
     Q/K packs are stored as fp32r (full-rate fp32, ~12-bit mantissa): the
     rounding cancels in the softmax ratio (verified ~1e-4 end-to-end).
  3. Score matmuls compute S^T [keys, queries] (K=4, 3x concurrent via PE
     row tiling tile_position=(32c,0)) into PSUM chunks [128, 3*512];
     one ACT Exp per chunk (scale=0.5 folds the softmax scale; no max
     subtraction needed, |score| <~ 15). ACT is the bottleneck engine.
  4. AV matmuls in bf16 (softmax-ratio error cancellation keeps end-to-end
     error ~2e-4) with a ones-augmented V (V~ [128, 5] per head) so softmax
     denominators accumulate in PSUM row 32c+4 for free; col tiling
     (tile_position=(0,32c)) packs 4 heads into one 2-bank PSUM accumulator
     (start=False onto DVE-memset PSUM). The chunk loop is software-
     pipelined one stage (scores+exp emitted before the previous chunk's
     AV) so the in-order PE queue never blocks the next exp. PSUM budget:
     2x3-bank score slots + 2-bank accumulator = all 8 banks.
  5. Tail (pipelined by query-half): one-hot const matmuls (fp32r) gather
     denominators (em) and reorder o rows (gm) into (h,d) order; DVE
     reciprocal + multiply normalizes; proj matmul; BN folded into one ACT
     Identity(scale,bias); unpool via masked multiplies (3 DVE + 1 GpSimd).
"""
import numpy as np

DIM = 64
HEAD_DIM = 4
NUM_HEADS = 16
B = 8
H = W = 64
HP = WP = 32
N = HP * WP          # 1024 tokens
NKT = 8              # key tiles of 128
BN_EPS = 1e-5

# Schraudolph exp (folds softmax scale 0.5), bf16/int16 flavor:
#   e^(0.5*s) ~= bitcast_bf16(int16(A_EXP*s + B_EXP))
A_EXP = float(0.5 * np.log2(np.e) * (1 << 7))
B_EXP = float(127 * (1 << 7) - 486411.0 / (1 << 16))
DVE_FRAC = 0.30      # fraction of exp chunks on DVE

_CACHE = {}


def _build_program():
    import concourse.bass as bass
    import concourse.mybir as mybir
    import concourse.tile as tile
    from concourse import bacc

    f32 = mybir.dt.float32
    f32r = mybir.dt.float32r
    bf16 = mybir.dt.bfloat16
    i16 = mybir.dt.int16
    AF = mybir.ActivationFunctionType
    OP = mybir.AluOpType

    nc = bacc.Bacc("TRN2", debug=False)

    x_d = nc.dram_tensor("x", [DIM, H * W], f32, kind="ExternalInput").ap()
    wa_d = nc.dram_tensor("wa", [128, 512], f32r, kind="ExternalInput").ap()
    wb_d = nc.dram_tensor("wb", [128, 1154], f32, kind="ExternalInput").ap()
    wc_d = nc.dram_tensor("wc", [128, 1024], f32r, kind="ExternalInput").ap()
    out_d = nc.dram_tensor("out", [DIM, H * W], f32, kind="ExternalOutput").ap()

    with tile.TileContext(nc) as tc:
        with (
            tc.tile_pool(name="singles", bufs=1) as sg1,
            tc.tile_pool(name="expp", bufs=4) as expp,
        ):
            # hoist the ACT exp-table load to t=0 via a dummy exp
            warm = sg1.tile([1, 1], f32)
            nc.vector.memset(warm, 0.0)
            nc.scalar.activation(warm, warm, AF.Exp)

            # ---------- loads (2 packed weight DMAs + x in 2 halves) ----------
            x_sb = sg1.tile([DIM, H * W], f32)
            xr = x_sb.rearrange("p (i ti j tj) -> p i ti j tj", ti=2, tj=2, j=WP)
            for qq in range(4):
                nc.sync.dma_start(out=x_sb[:, qq * 1024:(qq + 1) * 1024],
                                  in_=x_d[:, qq * 1024:(qq + 1) * 1024])
            wa_sb = sg1.tile([128, 512], f32r)
            nc.sync.dma_start(out=wa_sb, in_=wa_d)
            wb_sb = sg1.tile([128, 1154], f32)
            nc.sync.dma_start(out=wb_sb, in_=wb_d)
            wc_sb = sg1.tile([128, 1024], f32r)
            nc.sync.dma_start(out=wc_sb, in_=wc_d)
            em_sb = [wa_sb[:, 64 * sg:64 * sg + 64] for sg in range(4)]
            gm_sb = [wa_sb[:, 256 + 64 * sg:256 + 64 * sg + 64] for sg in range(4)]
            wqp_sb = [wc_sb[0:64, 128 * sg:128 * sg + 128] for sg in range(4)]
            wkp2_sb = [wc_sb[64:128, 512 + 128 * sg:512 + 128 * sg + 128]
                       for sg in range(4)]
            wv_sb = wb_sb[0:64, 1024:1088]
            wv2_sb = wb_sb[64:128, 1024:1088]
            wproj_sb = wb_sb[0:64, 1088:1152]
            bns_sb = wb_sb[0:64, 1152:1153]
            bnb_sb = wb_sb[0:64, 1153:1154]

            # ---------- maxpool (per x-half) ----------
            m01 = sg1.tile([DIM, N], f32)
            m23 = sg1.tile([DIM, N], f32)
            pooled = sg1.tile([DIM, N], f32)
            pooled_r = sg1.tile([DIM, N], f32r)
            m01r = m01.rearrange("p (i j) -> p i j", j=WP)
            m23r = m23.rearrange("p (i j) -> p i j", j=WP)
            pooledr = pooled.rearrange("p (i j) -> p i j", j=WP)
            v = [xr[:, :, 0, :, 0], xr[:, :, 0, :, 1],
                 xr[:, :, 1, :, 0], xr[:, :, 1, :, 1]]
            for hh in range(4):
                sl = slice(hh * 8, (hh + 1) * 8)
                nc.vector.tensor_tensor(m01r[:, sl], v[0][:, sl], v[1][:, sl], op=OP.max)
                nc.vector.tensor_tensor(m23r[:, sl], v[2][:, sl], v[3][:, sl], op=OP.max)
                nc.vector.tensor_tensor(pooledr[:, sl], m01r[:, sl], m23r[:, sl], op=OP.max)
                nc.scalar.copy(pooled_r[:, hh * 256:(hh + 1) * 256],
                               pooled[:, hh * 256:(hh + 1) * 256])

            # ---------- qkv packs + V~ (per pooled-half) ----------
            ones16 = sg1.tile([128, 16], f32)
            nc.vector.memset(ones16, 1.0)
            qtp = [sg1.tile([128, N], f32r, tag=f"qtp{sg}", name=f"qtp{sg}") for sg in range(4)]
            ktp = [sg1.tile([128, N], f32r, tag=f"ktp{sg}", name=f"ktp{sg}") for sg in range(4)]
            vt = [sg1.tile([128, 16, 5], bf16, tag=f"vt{kt}", name=f"vt{kt}") for kt in range(NKT)]
            pooled_r2 = sg1.tile([128, N], f32r)
            for qh in range(2):
                qsl = slice(qh * 512, (qh + 1) * 512)
                nc.vector.tensor_copy(pooled_r2[64:128, qsl], pooled[:, qsl])
            dummy_bf = sg1.tile([64, 512], bf16)
            nc.vector.memset(dummy_bf, 1.0)
            with (
                tc.tile_pool(name="prepq", bufs=6, space="PSUM") as prepq,
                tc.tile_pool(name="prepv", bufs=2, space="PSUM") as prepv,
            ):
                # PE warmup during the x DMA: back-to-back discarded matmuls
                # keep the HAM busy-window hot so prep matmuls run at 2.4GHz
                for wi in range(12):
                    w_ps = prepq.tile([128, 512], f32, tag="qkps")
                    nc.tensor.matmul(w_ps, dummy_bf[:, 0:128], dummy_bf,
                                     start=True, stop=True)
                for qh in range(2):
                    qsl = slice(qh * 512, (qh + 1) * 512)
                    for sg in range(4):
                        # q on row-strip 0 and k on strip 64 run concurrent;
                        # copies alternate ACT (early half) / DVE
                        cpq = nc.scalar.copy if qh == 0 else nc.vector.tensor_copy
                        cpk = nc.vector.tensor_copy
                        qt_ps = prepq.tile([128, 512], f32, tag="qkps")
                        nc.tensor.matmul(qt_ps, wqp_sb[sg], pooled_r[:, qsl],
                                         start=True, stop=True,
                                         tile_position=(0, 0))
                        kt_ps = prepq.tile([128, 512], f32, tag="qkps")
                        nc.tensor.matmul(kt_ps, wkp2_sb[sg],
                                         pooled_r2[64:128, qsl],
                                         start=True, stop=True,
                                         tile_position=(64, 0))
                        cpq(qtp[sg][:, qsl], qt_ps)
                        cpk(ktp[sg][:, qsl], kt_ps)
                    for kt in range(qh * 4, qh * 4 + 4):
                        v_ps = prepv.tile([128, DIM], f32, tag="vps")
                        nc.tensor.matmul(
                            v_ps, pooled[:, kt * 128:(kt + 1) * 128], wv_sb,
                            start=True, stop=True, tile_position=(0, 0))
                        nc.vector.tensor_copy(
                            vt[kt][:, :, 0:4],
                            v_ps.rearrange("p (h e) -> p h e", e=4))
                        nc.vector.tensor_copy(
                            vt[kt].rearrange("p h e -> p (h e)")[:, 4::5], ones16)

            # ---------- argmax masks (DVE, fills attention idle) ----------
            masks = []
            nf = None
            for p in range(4):
                eq = sg1.tile([DIM, N], f32, tag=f"eq{p}")
                eqr = eq.rearrange("p (i j) -> p i j", j=WP)
                nc.vector.tensor_tensor(eqr, v[p], pooledr, op=OP.is_equal)
                if p == 0:
                    masks.append(eq)
                    nf = sg1.tile([DIM, N], f32, tag="nf0")
                    nc.vector.tensor_scalar(nf, eq, -1.0, 1.0, op0=OP.mult, op1=OP.add)
                else:
                    mk = sg1.tile([DIM, N], f32, tag=f"mk{p}")
                    nc.vector.tensor_tensor(mk, eq, nf, op=OP.mult)
                    masks.append(mk)
                    if p < 3:
                        nf2 = sg1.tile([DIM, N], f32, tag=f"nf{p}")
                        nc.vector.tensor_tensor(nf2, nf, mk, op=OP.subtract)
                        nf = nf2

            # ---------- attention ----------
            chunks = [
                [(0, 0), (1, 0), (2, 0)],
                [(3, 0), (0, 1), (1, 1)],
                [(2, 1), (3, 1)],
            ]
            o_sb = [sg1.tile([128, N], f32r, tag=f"osb{sg}", name=f"osb{sg}") for sg in range(4)]
            dve_acc = [0.0]
            with (
                tc.tile_pool(name="spsum", bufs=2, space="PSUM") as spsum,
                tc.tile_pool(name="opsum", bufs=1, space="PSUM") as opsum,
            ):
                for sg in range(4):
                    o_ps = opsum.tile([128, N], f32, tag="ops")
                    nc.vector.memset(o_ps, 0.0)
                    # software-pipeline by one chunk: emit scores+exp, then
                    # the PREVIOUS chunk's AV matmuls, so PE's in-order queue
                    # never parks AV work in front of the next chunk's scores
                    pend = None

                    def flush_av(pend):
                        kt0, ch0, e0 = pend
                        for i, (c, qh) in enumerate(ch0):
                            nc.tensor.matmul(
                                o_ps[32 * c:32 * c + 5,
                                     qh * 512:(qh + 1) * 512],
                                vt[kt0][:, 4 * sg + c, :],
                                e0[:, i * 512:(i + 1) * 512],
                                start=False, stop=(kt0 == NKT - 1),
                                skip_group_check=True,
                                tile_position=(0, 32 * c))

                    for kt in range(NKT):
                        for ch in chunks:
                            ncb = len(ch)
                            s_ps = spsum.tile([128, 3 * 512], f32, tag="slot")
                            for i, (c, qh) in enumerate(ch):
                                nc.tensor.matmul(
                                    s_ps[:, i * 512:(i + 1) * 512],
                                    ktp[sg][32 * c:32 * c + 4,
                                            kt * 128:(kt + 1) * 128],
                                    qtp[sg][32 * c:32 * c + 4,
                                            qh * 512:(qh + 1) * 512],
                                    start=True, stop=True,
                                    tile_position=(32 * c, 0))
                            e_sb = expp.tile([128, 3 * 512], bf16, tag="exp")
                            dve_acc[0] += DVE_FRAC
                            if dve_acc[0] >= 1.0:
                                dve_acc[0] -= 1.0
                                nc.vector.tensor_scalar(
                                    e_sb.bitcast(i16)[:, :ncb * 512],
                                    s_ps[:, :ncb * 512],
                                    A_EXP, B_EXP, op0=OP.mult, op1=OP.add)
                            else:
                                nc.scalar.activation(
                                    e_sb[:, :ncb * 512], s_ps[:, :ncb * 512],
                                    AF.Exp, scale=0.5)
                            if pend is not None:
                                flush_av(pend)
                            pend = (kt, ch, e_sb)
                    flush_av(pend)
                    # evict accumulator (o_ps fully initialized via memset)
                    nc.vector.tensor_copy(o_sb[sg][:, 0:512], o_ps[:, 0:512])
                    nc.vector.tensor_copy(o_sb[sg][:, 512:1024], o_ps[:, 512:1024])

            # ---------- tail: per-half pipeline normalize+proj+bn+unpool ----------
            out_sb = sg1.tile([DIM, H * W], f32)
            outr = out_sb.rearrange("p (i ti j tj) -> p i ti j tj",
                                    ti=2, tj=2, j=WP)
            yr_all = []
            with tc.tile_pool(name="tailps", bufs=1, space="PSUM") as tailps:
                dr = sg1.tile([DIM, N], f32)
                onorm = sg1.tile([DIM, N], f32)
                y = sg1.tile([DIM, N], f32)
                for qh in range(2):
                    qsl = slice(qh * 512, (qh + 1) * 512)
                    d_ps = tailps.tile([DIM, 512], f32, tag=f"dps{qh}")
                    o2_ps = tailps.tile([DIM, 512], f32, tag=f"o2ps{qh}")
                    for sg in range(4):
                        nc.tensor.matmul(
                            d_ps, em_sb[sg], o_sb[sg][:, qsl],
                            start=(sg == 0), stop=(sg == 3))
                    for sg in range(4):
                        nc.tensor.matmul(
                            o2_ps, gm_sb[sg], o_sb[sg][:, qsl],
                            start=(sg == 0), stop=(sg == 3))
                    nc.vector.reciprocal(dr[:, qsl], d_ps)
                    nc.vector.tensor_tensor(onorm[:, qsl], o2_ps, dr[:, qsl],
                                            op=OP.mult)
                    pj_ps = tailps.tile([DIM, 512], f32, tag=f"pjps{qh}")
                    nc.tensor.matmul(
                        pj_ps, wproj_sb, onorm[:, qsl],
                        start=True, stop=True)
                    nc.scalar.activation(
                        y[:, qsl], pj_ps, AF.Identity, bias=bnb_sb, scale=bns_sb)
                    yr = y.rearrange("p (i j) -> p i j", j=WP)
                    sl = slice(qh * 16, (qh + 1) * 16)
                    for p in range(4):
                        mr = masks[p].rearrange("p (i j) -> p i j", j=WP)
                        eng = nc.vector if p < 3 else nc.gpsimd
                        eng.tensor_tensor(
                            outr[:, sl, p // 2, :, p % 2], yr[:, sl], mr[:, sl],
                            op=OP.mult)
                    nc.sync.dma_start(
                        out=out_d[:, qh * 2048:(qh + 1) * 2048],
                        in_=out_sb[:, qh * 2048:(qh + 1) * 2048])

    nc.compile()
    return nc


def _host_inputs(x, w_qkv, w_proj, gamma, beta, bn_mean, bn_var):
    """Build the per-core input maps (host-side packing)."""
    wq = w_qkv[:, 0:64]
    wk = w_qkv[:, 64:128]
    wv = np.ascontiguousarray(w_qkv[:, 128:192], dtype=np.float32)
    wqp = np.zeros((4, DIM, 128), np.float32)
    wkp = np.zeros((4, DIM, 128), np.float32)
    em = np.zeros((4, 128, DIM), np.float32)
    gm = np.zeros((4, 128, DIM), np.float32)
    for sg in range(4):
        for c in range(4):
            h = 4 * sg + c
            for d in range(HEAD_DIM):
                wqp[sg][:, 32 * c + d] = wq[:, 4 * h + d]
                wkp[sg][:, 32 * c + d] = wk[:, 4 * h + d]
                gm[sg][32 * c + d, 4 * h + d] = 1.0
                em[sg][32 * c + 4, 4 * h + d] = 1.0
    inv = gamma / np.sqrt(bn_var + BN_EPS)
    bns = inv.reshape(DIM, 1).astype(np.float32)
    bnb = (beta - bn_mean * inv).reshape(DIM, 1).astype(np.float32)
    wproj = np.ascontiguousarray(w_proj, dtype=np.float32)

    wa = np.zeros((128, 512), np.float32)
    for sg in range(4):
        wa[:, 64 * sg:64 * sg + 64] = em[sg]
        wa[:, 256 + 64 * sg:256 + 64 * sg + 64] = gm[sg]
    wb = np.zeros((128, 1154), np.float32)
    for sg in range(4):
        wb[0:64, 128 * sg:128 * sg + 128] = wqp[sg]
        wb[0:64, 512 + 128 * sg:512 + 128 * sg + 128] = wkp[sg]
    wb[0:64, 1024:1088] = wv
    wb[0:64, 1088:1152] = wproj
    wb[0:64, 1152:1153] = bns
    wb[0:64, 1153:1154] = bnb
    wb[64:128, :] = wb[0:64, :]
    wc = np.ascontiguousarray(wb[:, 0:1024])
    shared = {"wa": wa, "wb": wb, "wc": wc}
    in_maps = []
    for b in range(B):
        m = dict(shared)
        m["x"] = np.ascontiguousarray(
            np.asarray(x)[b].reshape(DIM, H * W), dtype=np.float32)
        in_maps.append(m)
    return in_maps


def kernel(x, w_qkv, w_proj, gamma, beta, bn_mean, bn_var):
    from concourse import bass_utils

    if "nc" not in _CACHE:
        _CACHE["nc"] = _build_program()
    nc = _CACHE["nc"]
    in_maps = _host_inputs(
        np.asarray(x), np.asarray(w_qkv), np.asarray(w_proj),
        np.asarray(gamma), np.asarray(beta),
        np.asarray(bn_mean), np.asarray(bn_var))
    res = bass_utils.run_bass_kernel_spmd(nc, in_maps, core_ids=list(range(B)))
    out = np.stack([res.results[b]["out"].reshape(DIM, H, W) for b in range(B)])
    return out.astype(np.float32)

